# revision 11
# baseline (speedup 1.0000x reference)
"""Trainium2 Bass kernel for MultiHead GQA attention (B=1, S=2048, D=1024,
16 q-heads / 4 kv-heads, GQA group 4, RoPE, causal).

Sharding: tensor-parallel over heads. Core c (of 8) computes 2 query heads
{g, g+4} (c even) or {g+8, g+12} (c odd) with g = c//2, which all attend kv
head g (jnp.tile GQA semantics: q-head h uses kv head h % 4). Wq/Wk/Wv are
column-sharded, Wo row-sharded; each core produces a partial [D, S] output
(transposed) and the host reduces the 8 partials, transposes, and adds bo.

Device dataflow (per core, everything in "transposed" [feature, seq] layout
so no on-device transposes of activations are needed):
  qhT[128,S]  = Wq_c.T @ qT   (+bq)   -> RoPE (block-deinterleaved pairs)
  khT[64,S]   = Wk_c.T @ kT   (+bk)   -> RoPE
  vhT[64,S]   = Wv_c.T @ vT   (+bv)   -> PE-transposed to vh[S,64] (+ ones col)
  scoresT[j,i] = khT[:,j].T @ qhT[:,i]          (PE, K=64)
  pT = exp(scoresT/8)       (ACT, causal-masked via precomputed 0/1 tiles)
  o_aug[65,i] += vh_aug[j].T @ pT[j,i]          (PE; row 64 = softmax denom)
  norm: recip of denom (DVE) -> PE broadcast over 64 partitions -> DVE mul
  outT_partial[e,s] = Wo_c[:,e].T @ attnT       (PE)

RoPE trick: the head_dim is permuted on the host (even dims then odd dims)
in Wq/Wk columns, so rotation pairs are partition blocks [0:32)/[32:64) and
the device applies rope with quadrant-aligned copies + two muls + add using
host-precomputed cos / sign-folded sin tables. The permutation cancels in
q.k dot products and does not touch V or Wo.

The causal mask input is not transferred: the reference mask is tril(ones)
and masking is applied structurally (skipped tiles + 4 precomputed diagonal
mask tiles).
"""

import numpy as np
from contextlib import ExitStack

import concourse.bass as bass
from concourse import bacc
import concourse.mybir as mybir
import concourse.tile as tile
from concourse.bass_utils import run_bass_kernel_spmd
from concourse.masks import make_identity

f32 = mybir.dt.float32

S = 2048
D = 1024
HEADS = 16
HD = 64
KVH = 4
N_CORES = 8

ST = 512          # i-tile (free dim of most matmuls)
NS = S // ST      # 4
FP = 128          # contraction chunk
NF = D // FP      # 8
JTS = 128         # j-chunk (key positions per score tile partition dim)
NJ = S // JTS     # 16
NE = D // 128     # 8 output-feature chunks

_CACHE = {}


def _build_program():
    if "nc" in _CACHE:
        return _CACHE["nc"]

    nc = bacc.Bacc("TRN2", target_bir_lowering=False, debug=False)

    def din(name, shape):
        return nc.dram_tensor(name, shape, f32, kind="ExternalInput").ap()

    qT = din("qT", [D, S])
    kT = din("kT", [D, S])
    vT = din("vT", [D, S])
    wq = din("wq", [D, 128])
    wk = din("wk", [D, 64])
    wv = din("wv", [D, 64])
    wo = din("wo", [128, D])
    bq = din("bq", [128, 1])
    bk = din("bk", [64, 1])
    bv = din("bv", [64, 1])
    cosk = din("cosk", [64, S])
    sink = din("sink", [64, S])
    cmask = din("cmask", [128, 896])
    outT = nc.dram_tensor("outT", [D, S], f32, kind="ExternalOutput").ap()

    Identity = mybir.ActivationFunctionType.Identity
    Exp = mybir.ActivationFunctionType.Exp

    with tile.TileContext(nc) as tc, ExitStack() as ctx:
        const = ctx.enter_context(tc.tile_pool(name="const", bufs=1))
        big = ctx.enter_context(tc.tile_pool(name="big", bufs=1))
        stream = ctx.enter_context(tc.tile_pool(name="stream", bufs=3))
        ptile = ctx.enter_context(tc.tile_pool(name="ptile", bufs=4))
        small = ctx.enter_context(tc.tile_pool(name="small", bufs=8))
        outb = ctx.enter_context(tc.tile_pool(name="outb", bufs=3))
        psum = ctx.enter_context(tc.tile_pool(name="psum", bufs=4, space="PSUM"))

        # scratch targets for "touch" ops: a touch makes an engine observe a
        # DMA-queue (or other engine's) semaphore once, so later real
        # instructions carry at most ONE sync wait (HW codegen limit on PE).
        scr = psum.tile([128, 1], f32, tag="scr", bufs=1)
        scr_act = const.tile([128, 4], f32, tag="scr_act", name="scr_act")
        scr_dve = const.tile([128, 4], f32, tag="scr_dve", name="scr_dve")

        def pe_touch(ap2d):
            nc.tensor.matmul(scr[0:1, 0:1], lhsT=ap2d, rhs=ap2d,
                             start=True, stop=True)

        # ---- constants ----
        ident = const.tile([128, 128], f32)
        make_identity(nc, ident)
        pe_touch(ident[:, 0:1])
        ones64 = const.tile([1, 64], f32)
        nc.vector.memset(ones64, 1.0)
        # sliding causal mask: cm_sb[jp, c] = 1.0 iff jp <= c - 384
        # diagonal tile o (= jt - 4*it) uses slice [384-128*o : 896-128*o]
        cm_sb = const.tile([128, 896], f32)
        nc.sync.dma_start(out=cm_sb, in_=cmask)

        wq_sb = const.tile([128, NF, 128], f32)
        nc.sync.dma_start(out=wq_sb, in_=wq.rearrange("(nf fp) d -> fp nf d", fp=FP))
        wk_sb = const.tile([128, NF, 64], f32)
        nc.sync.dma_start(out=wk_sb, in_=wk.rearrange("(nf fp) d -> fp nf d", fp=FP))
        wv_sb = const.tile([128, NF, 64], f32)
        nc.sync.dma_start(out=wv_sb, in_=wv.rearrange("(nf fp) d -> fp nf d", fp=FP))
        wo_sb = const.tile([128, D], f32)
        nc.sync.dma_start(out=wo_sb, in_=wo)
        bq_sb = const.tile([128, 1], f32)
        nc.sync.dma_start(out=bq_sb, in_=bq)
        bk_sb = const.tile([64, 1], f32)
        nc.sync.dma_start(out=bk_sb, in_=bk)
        bv_sb = const.tile([64, 1], f32)
        nc.sync.dma_start(out=bv_sb, in_=bv)
        cosk_sb = const.tile([64, S], f32)
        nc.sync.dma_start(out=cosk_sb, in_=cosk)
        sink_sb = const.tile([64, S], f32)
        nc.sync.dma_start(out=sink_sb, in_=sink)

        # observe const DMA queues on their consuming engines
        pe_touch(wq_sb[:, 0, 0:1])
        pe_touch(wk_sb[:, 0, 0:1])
        pe_touch(wv_sb[:, 0, 0:1])
        pe_touch(wo_sb[:, 0:1])
        nc.scalar.copy(scr_act[0:128, 0:1], bq_sb)
        nc.scalar.copy(scr_act[0:64, 1:2], bk_sb)
        nc.scalar.copy(scr_act[0:64, 2:3], bv_sb)
        nc.vector.tensor_copy(scr_dve[0:64, 0:1], cosk_sb[:, 0:1])
        nc.vector.tensor_copy(scr_dve[0:64, 1:2], sink_sb[:, 0:1])
        nc.vector.tensor_copy(scr_dve[0:128, 2:3], cm_sb[:, 0:1])

        # ---- projections ----
        def project(src_dram, w_sb, nd, copies):
            # copies: list of (psum_row_base, dst_ap_fn, bias_ap) drains
            ps = [psum.tile([128, ST], f32, tag="acc", bufs=4, name=f"pj{s}") for s in range(NS)]
            for f in range(NF):
                xin = stream.tile([128, S], f32, tag="xin")
                nc.sync.dma_start(out=xin, in_=src_dram[f * FP:(f + 1) * FP, :])
                pe_touch(xin[:, 0:1])
                for s in range(NS):
                    nc.tensor.matmul(
                        ps[s][0:nd, :], lhsT=w_sb[:, f, 0:nd],
                        rhs=xin[:, s * ST:(s + 1) * ST],
                        start=(f == 0), stop=(f == NF - 1),
                    )
            for s in range(NS):
                for (r0, r1, dst, bias_ap) in copies:
                    nc.scalar.activation(
                        out=dst[:, s * ST:(s + 1) * ST], in_=ps[s][r0:r1, :],
                        func=Identity, bias=bias_ap,
                    )

        qh0 = big.tile([64, S], f32)
        qh1 = big.tile([64, S], f32)
        khT = big.tile([64, S], f32)
        vhT = big.tile([64, S], f32)
        project(qT, wq_sb, 128,
                [(0, 64, qh0, bq_sb[0:64, :]), (64, 128, qh1, bq_sb[64:128, :])])
        project(kT, wk_sb, 64, [(0, 64, khT, bk_sb)])
        project(vT, wv_sb, 64, [(0, 64, vhT, bv_sb)])

        # ---- RoPE (in-place; pairs are partition blocks [0:32)/[32:64)) ----
        def rope64(x, nm):
            swap = stream.tile([64, S], f32, tag="swap", name=f"swap_{nm}")
            for (srcp, dstp) in ((32, 0), (0, 32)):
                nc.vector.tensor_copy(swap[dstp:dstp + 32, :], x[srcp:srcp + 32, :])
            nc.vector.tensor_mul(x, x, cosk_sb)
            nc.vector.tensor_mul(swap, swap, sink_sb)
            nc.vector.tensor_add(x, x, swap)

        rope64(qh0, "q0")
        rope64(qh1, "q1")
        rope64(khT, "k")
        pe_touch(qh0[:, 0:1])
        pe_touch(qh1[:, 0:1])
        pe_touch(khT[:, 0:1])

        # ---- transpose V to [seq, dim] (+ ones column for softmax denom) ----
        vh_aug = big.tile([128, NJ, 65], f32)
        nc.vector.memset(vh_aug[:, :, 64:65], 1.0)
        for jt in range(NJ):
            tp = psum.tile([128, 64], f32, tag="mm", bufs=3, name="tp")
            nc.tensor.transpose(tp, vhT[:, jt * JTS:(jt + 1) * JTS], ident[0:64, 0:64])
            nc.scalar.copy(vh_aug[:, jt, 0:64], tp)

        # ---- attention (2 heads share khT / vh_aug) ----
        attn = big.tile([128, S], f32)
        for h in range(2):
            q_sl = (qh0, qh1)[h]
            po = [psum.tile([65, ST], f32, tag="acc", bufs=4, name=f"po{s}") for s in range(NS)]
            for jt in range(NJ):
                it0 = jt // 4
                for it in range(it0, NS):
                    ps = psum.tile([128, ST], f32, tag="mm", bufs=3)
                    nc.tensor.matmul(
                        ps, lhsT=khT[:, jt * JTS:(jt + 1) * JTS],
                        rhs=q_sl[:, it * ST:(it + 1) * ST],
                        start=True, stop=True,
                    )
                    pt = ptile.tile([128, ST], f32, tag="pt")
                    nc.scalar.activation(out=pt, in_=ps, func=Exp, scale=0.125)
                    if it == it0:
                        o = jt - 4 * it0
                        nc.vector.tensor_mul(
                            pt, pt, cm_sb[:, 384 - 128 * o:896 - 128 * o])
                    nc.tensor.matmul(
                        po[it], lhsT=vh_aug[:, jt, :], rhs=pt,
                        start=(jt == 0), stop=(jt == 4 * it + 3),
                    )
            for it in range(NS):
                a_sl = attn[h * 64:(h + 1) * 64, it * ST:(it + 1) * ST]
                nc.scalar.copy(a_sl, po[it][0:64, :])
                sums = small.tile([1, ST], f32, tag="sums")
                nc.scalar.copy(sums, po[it][64:65, :])
                rc = small.tile([1, ST], f32, tag="rc")
                nc.vector.reciprocal(rc, sums)
                pe_touch(rc[:, 0:1])
                bcp = psum.tile([64, ST], f32, tag="mm", bufs=3, name="bcp")
                nc.tensor.matmul(bcp, lhsT=ones64, rhs=rc, start=True, stop=True)
                nc.vector.tensor_mul(a_sl, a_sl, bcp)

        # ---- output projection (partial over this core's 128 dims) ----
        for e in range(NE):
            for it in range(NS):
                pw = psum.tile([128, ST], f32, tag="mm", bufs=3, name="pw")
                nc.tensor.matmul(
                    pw, lhsT=wo_sb[:, e * 128:(e + 1) * 128],
                    rhs=attn[:, it * ST:(it + 1) * ST],
                    start=True, stop=True,
                )
                ob = outb.tile([128, ST], f32, tag="ob")
                nc.vector.tensor_copy(ob, pw)
                nc.sync.dma_start(
                    out=outT[e * 128:(e + 1) * 128, it * ST:(it + 1) * ST], in_=ob
                )

    nc.compile()
    _CACHE["nc"] = nc
    return nc


def _host_tables():
    if "tables" in _CACHE:
        return _CACHE["tables"]
    # faithful to reference: exp = -2*arange(0,64,2)/64
    expv = -2.0 * np.arange(0, HD, 2, dtype=np.float32) / HD
    thetas = np.power(np.float32(10000.0), expv).astype(np.float32)    # [32]
    m = np.arange(S, dtype=np.float32)
    freq = np.outer(m, thetas).astype(np.float32)                      # [S, 32]
    cos = np.cos(freq).astype(np.float32).T                            # [32, S]
    sin = np.sin(freq).astype(np.float32).T
    cos64 = np.concatenate([cos, cos], 0)                              # [64, S]
    sin64 = np.concatenate([-sin, sin], 0)                             # [64, S]
    cos64 = np.ascontiguousarray(cos64)
    sin64 = np.ascontiguousarray(sin64)
    perm = np.concatenate([np.arange(0, HD, 2), np.arange(1, HD, 2)])  # deinterleave
    slide = (np.arange(128)[:, None] <= (np.arange(896)[None, :] - 384))
    slide = np.ascontiguousarray(slide.astype(np.float32))
    _CACHE["tables"] = (cos64, sin64, perm, slide)
    return _CACHE["tables"]


def kernel(**inputs):
    q = np.asarray(inputs["q"], np.float32)[0]       # [S, D]
    k = np.asarray(inputs["k"], np.float32)[0]
    v = np.asarray(inputs["v"], np.float32)[0]
    Wq = np.asarray(inputs["Wq"], np.float32)
    Wk = np.asarray(inputs["Wk"], np.float32)
    Wv = np.asarray(inputs["Wv"], np.float32)
    Wo = np.asarray(inputs["Wo"], np.float32)
    bq = np.asarray(inputs["bq"], np.float32)
    bk = np.asarray(inputs["bk"], np.float32)
    bv = np.asarray(inputs["bv"], np.float32)
    bo = np.asarray(inputs["bo"], np.float32)

    cos64, sin64, perm, slide = _host_tables()

    # head_dim deinterleave permutation applied to q/k projection columns
    permQ = np.concatenate([h * HD + perm for h in range(HEADS)])
    permK = np.concatenate([g * HD + perm for g in range(KVH)])
    Wqp = Wq[:, permQ]
    bqp = bq[permQ]
    Wkp = Wk[:, permK]
    bkp = bk[permK]

    qT = np.ascontiguousarray(q.T)
    kT = np.ascontiguousarray(k.T)
    vT = np.ascontiguousarray(v.T)

    in_maps = []
    head_pairs = []
    for c in range(N_CORES):
        g = c // 2
        if c % 2 == 0:
            h0, h1 = g, g + 4
        else:
            h0, h1 = g + 8, g + 12
        head_pairs.append((h0, h1))
        wq_c = np.ascontiguousarray(
            np.concatenate([Wqp[:, h0 * HD:(h0 + 1) * HD],
                            Wqp[:, h1 * HD:(h1 + 1) * HD]], axis=1))
        bq_c = np.ascontiguousarray(
            np.concatenate([bqp[h0 * HD:(h0 + 1) * HD],
                            bqp[h1 * HD:(h1 + 1) * HD]]).reshape(128, 1))
        wo_c = np.ascontiguousarray(
            np.concatenate([Wo[h0 * HD:(h0 + 1) * HD, :],
                            Wo[h1 * HD:(h1 + 1) * HD, :]], axis=0))
        in_maps.append({
            "qT": qT, "kT": kT, "vT": vT,
            "wq": wq_c,
            "wk": np.ascontiguousarray(Wkp[:, g * HD:(g + 1) * HD]),
            "wv": np.ascontiguousarray(Wv[:, g * HD:(g + 1) * HD]),
            "wo": wo_c,
            "bq": bq_c,
            "bk": np.ascontiguousarray(bkp[g * HD:(g + 1) * HD].reshape(64, 1)),
            "bv": np.ascontiguousarray(bv[g * HD:(g + 1) * HD].reshape(64, 1)),
            "cosk": cos64, "sink": sin64, "cmask": slide,
        })

    nc = _build_program()
    res = run_bass_kernel_spmd(nc, in_maps, list(range(N_CORES)))
    acc = np.zeros((D, S), np.float32)
    for r in res.results:
        acc += np.asarray(r["outT"], np.float32)
    out = acc.T + bo[None, :]
    return out[None].astype(np.float32)


# revision 24
# speedup vs baseline: 1.1682x; 1.1682x over previous
"""Trainium2 Bass kernel for MultiHead GQA attention (B=1, S=2048, D=1024,
16 q-heads / 4 kv-heads, GQA group 4, RoPE, causal).

Sharding: tensor-parallel over heads. Core c (of 8) computes 2 query heads
{g, g+4} (c even) or {g+8, g+12} (c odd) with g = c//2, which all attend kv
head g (jnp.tile GQA semantics: q-head h uses kv head h % 4). Wq/Wk/Wv are
column-sharded, Wo row-sharded; each core produces a partial [D, S] output
(transposed) and the host reduces the 8 partials, transposes, and adds bo.

Device dataflow (per core, everything in "transposed" [feature, seq] layout
so no on-device transposes of activations are needed):
  qhT[128,S]  = Wq_c.T @ qT   (+bq)   -> RoPE (block-deinterleaved pairs)
  khT[64,S]   = Wk_c.T @ kT   (+bk)   -> RoPE
  vhT[64,S]   = Wv_c.T @ vT   (+bv)   -> PE-transposed to vh[S,64] (+ ones col)
  scoresT[j,i] = khT[:,j].T @ qhT[:,i]          (PE, K=64)
  pT = exp(scoresT/8)       (ACT, causal-masked via precomputed 0/1 tiles)
  o_aug[65,i] += vh_aug[j].T @ pT[j,i]          (PE; row 64 = softmax denom)
  norm: recip of denom (DVE) -> PE broadcast over 64 partitions -> DVE mul
  outT_partial[e,s] = Wo_c[:,e].T @ attnT       (PE)

RoPE trick: the head_dim is permuted on the host (even dims then odd dims)
in Wq/Wk columns, so rotation pairs are partition blocks [0:32)/[32:64) and
the device applies rope with quadrant-aligned copies + two muls + add using
host-precomputed cos / sign-folded sin tables. The permutation cancels in
q.k dot products and does not touch V or Wo.

The causal mask input is not transferred: the reference mask is tril(ones)
and masking is applied structurally (skipped tiles + 4 precomputed diagonal
mask tiles).
"""

import numpy as np
from contextlib import ExitStack

import concourse.bass as bass
from concourse import bacc
import concourse.mybir as mybir
import concourse.tile as tile
from concourse.bass_utils import run_bass_kernel_spmd

f32 = mybir.dt.float32
f32r = mybir.dt.float32r
USE_F32R = True
MDT = f32r if USE_F32R else f32

S = 2048
D = 1024
HEADS = 16
HD = 64
KVH = 4
N_CORES = 8

ST = 512          # i-tile (free dim of most matmuls)
NS = S // ST      # 4
FP = 128          # contraction chunk
NF = D // FP      # 8
JTS = 128         # j-chunk (key positions per score tile partition dim)
NJ = S // JTS     # 16
NE = D // 128     # 8 output-feature chunks

_CACHE = {}


def _build_program():
    if "nc" in _CACHE:
        return _CACHE["nc"]

    nc = bacc.Bacc("TRN2", target_bir_lowering=False, debug=False)

    def din(name, shape, dt=f32):
        return nc.dram_tensor(name, shape, dt, kind="ExternalInput").ap()

    qT = din("qT", [D, S], MDT)
    kT = din("kT", [D, S], MDT)
    vT = din("vT", [D, S], MDT)
    wq = din("wq", [D, 128], MDT)
    wk = din("wk", [D, 64], MDT)
    wv = din("wv", [D, 64], MDT)
    wo = din("wo", [128, D], MDT)
    bq = din("bq", [128, 1])
    bk = din("bk", [64, 1])
    bv = din("bv", [64, 1])
    cosk = din("cosk", [64, S], MDT)
    sink = din("sink", [64, S], MDT)
    cmask = din("cmask", [128, 896], MDT)
    ident_in = din("ident", [64, 64], MDT)
    ones_in = din("ones", [128, 64], MDT)
    outT = nc.dram_tensor("outT", [D, S], f32, kind="ExternalOutput").ap()

    Identity = mybir.ActivationFunctionType.Identity
    Exp = mybir.ActivationFunctionType.Exp

    with tile.TileContext(nc) as tc, ExitStack() as ctx:
        const = ctx.enter_context(tc.tile_pool(name="const", bufs=1))
        big = ctx.enter_context(tc.tile_pool(name="big", bufs=1))
        stream = ctx.enter_context(tc.tile_pool(name="stream", bufs=3))
        ptile = ctx.enter_context(tc.tile_pool(name="ptile", bufs=4))
        small = ctx.enter_context(tc.tile_pool(name="small", bufs=8))
        outb = ctx.enter_context(tc.tile_pool(name="outb", bufs=3))
        psum = ctx.enter_context(tc.tile_pool(name="psum", bufs=4, space="PSUM"))

        # scratch targets for "touch" ops: a touch makes an engine observe a
        # DMA-queue (or other engine's) semaphore once, so later real
        # instructions carry at most ONE sync wait (HW codegen limit on PE).
        scr = psum.tile([128, 1], f32, tag="scr", bufs=1)
        scr_act = const.tile([128, 4], f32, tag="scr_act", name="scr_act")
        scr_dve = const.tile([128, 4], f32, tag="scr_dve", name="scr_dve")

        def pe_touch(ap2d):
            ap2d = ap2d.bitcast(f32)
            nc.tensor.matmul(scr[0:1, 0:1], lhsT=ap2d, rhs=ap2d,
                             start=True, stop=True)

        def mm(out, lhsT, rhs, start, stop):
            nc.tensor.matmul(out, lhsT=lhsT, rhs=rhs, start=start, stop=stop)

        # ---- constants ----
        ident = const.tile([64, 64], MDT)
        nc.sync.dma_start(out=ident, in_=ident_in)
        pe_touch(ident[:, 0:1])
        ones4q = const.tile([128, 64], MDT)
        nc.sync.dma_start(out=ones4q, in_=ones_in)
        pe_touch(ones4q[:, 0:1])
        # sliding causal mask: cm_sb[jp, c] = 1.0 iff jp <= c - 384
        # diagonal tile o (= jt - 4*it) uses slice [384-128*o : 896-128*o]
        cm_sb = const.tile([128, 896], MDT)
        nc.sync.dma_start(out=cm_sb, in_=cmask)

        wq_sb = const.tile([128, NF, 128], MDT)
        nc.sync.dma_start(out=wq_sb, in_=wq.rearrange("(nf fp) d -> fp nf d", fp=FP))
        wk_sb = const.tile([128, NF, 64], MDT)
        nc.sync.dma_start(out=wk_sb, in_=wk.rearrange("(nf fp) d -> fp nf d", fp=FP))
        wv_sb = const.tile([128, NF, 64], MDT)
        nc.sync.dma_start(out=wv_sb, in_=wv.rearrange("(nf fp) d -> fp nf d", fp=FP))
        wo_sb = const.tile([128, D], MDT)
        nc.sync.dma_start(out=wo_sb, in_=wo)
        bq_sb = const.tile([128, 1], f32)
        nc.sync.dma_start(out=bq_sb, in_=bq)
        bk_sb = const.tile([64, 1], f32)
        nc.sync.dma_start(out=bk_sb, in_=bk)
        bv_sb = const.tile([64, 1], f32)
        nc.sync.dma_start(out=bv_sb, in_=bv)
        cosk_sb = const.tile([64, S], MDT)
        nc.sync.dma_start(out=cosk_sb, in_=cosk)
        sink_sb = const.tile([64, S], MDT)
        nc.sync.dma_start(out=sink_sb, in_=sink)

        # observe const DMA queues on their consuming engines
        pe_touch(wq_sb[:, 0, 0:1])
        pe_touch(wk_sb[:, 0, 0:1])
        pe_touch(wv_sb[:, 0, 0:1])
        pe_touch(wo_sb[:, 0:1])
        nc.scalar.copy(scr_act[0:128, 0:1], bq_sb)
        nc.scalar.copy(scr_act[0:64, 1:2], bk_sb)
        nc.scalar.copy(scr_act[0:64, 2:3], bv_sb)
        nc.vector.tensor_copy(scr_dve[0:64, 0:1], cosk_sb[:, 0:1])
        nc.vector.tensor_copy(scr_dve[0:64, 1:2], sink_sb[:, 0:1])
        nc.vector.tensor_copy(scr_dve[0:128, 2:3], cm_sb[:, 0:1])

        # ---- projections ----
        def project(src_dram, w_sb, nd, copies):
            # copies: list of (psum_row_base, dst_ap_fn, bias_ap) drains
            ps = [psum.tile([128, ST], f32, tag="acc", bufs=4, name=f"pj{s}") for s in range(NS)]
            for f in range(NF):
                xin = stream.tile([128, S], MDT, tag="xin")
                nc.sync.dma_start(out=xin, in_=src_dram[f * FP:(f + 1) * FP, :])
                pe_touch(xin[:, 0:1])
                for s in range(NS):
                    mm(ps[s][0:nd, :], w_sb[:, f, 0:nd],
                       xin[:, s * ST:(s + 1) * ST],
                       start=(f == 0), stop=(f == NF - 1))
            for s in range(NS):
                for (r0, r1, dst, bias_ap) in copies:
                    nc.scalar.activation(
                        out=dst[:, s * ST:(s + 1) * ST], in_=ps[s][r0:r1, :],
                        func=Identity, bias=bias_ap,
                    )

        qh0 = big.tile([64, S], MDT)
        qh1 = big.tile([64, S], MDT)
        khT = big.tile([64, S], MDT)
        vhT = big.tile([64, S], MDT)
        project(qT, wq_sb, 128,
                [(0, 64, qh0, bq_sb[0:64, :]), (64, 128, qh1, bq_sb[64:128, :])])
        project(kT, wk_sb, 64, [(0, 64, khT, bk_sb)])
        project(vT, wv_sb, 64, [(0, 64, vhT, bv_sb)])

        # ---- RoPE (in-place; pairs are partition blocks [0:32)/[32:64)) ----
        def rope64(x, nm):
            swap = stream.tile([64, S], MDT, tag="swap", name=f"swap_{nm}")
            for (srcp, dstp) in ((32, 0), (0, 32)):
                nc.vector.tensor_copy(swap[dstp:dstp + 32, :], x[srcp:srcp + 32, :])
            nc.vector.tensor_mul(x, x, cosk_sb)
            nc.vector.tensor_mul(swap, swap, sink_sb)
            nc.vector.tensor_add(x, x, swap)

        rope64(qh0, "q0")
        rope64(qh1, "q1")
        rope64(khT, "k")
        pe_touch(qh0[:, 0:1])
        pe_touch(qh1[:, 0:1])
        pe_touch(khT[:, 0:1])

        # ---- transpose V to [seq, dim] (+ ones column for softmax denom) ----
        vh_aug = big.tile([128, NJ, 65], MDT)
        nc.scalar.copy(vh_aug[:, :, 64], ones4q[0:128, 0:NJ])
        with nc.allow_low_precision(reason="f32r transpose psum, same width as f32"):
            for jt in range(NJ):
                tp = psum.tile([128, 64], MDT, tag="mm", bufs=3, name="tp")
                nc.tensor.transpose(tp, vhT[:, jt * JTS:(jt + 1) * JTS],
                                    ident)
                nc.scalar.copy(vh_aug[:, jt, 0:64], tp)

        # ---- attention (2 heads share khT / vh_aug) ----
        attn = big.tile([128, S], MDT)
        for h in range(2):
            q_sl = (qh0, qh1)[h]
            po = [psum.tile([65, ST], f32, tag="acc", bufs=4, name=f"po{s}") for s in range(NS)]
            for jt in range(NJ):
                it0 = jt // 4
                for it in range(it0, NS):
                    ps = psum.tile([128, ST], f32, tag="mm", bufs=3)
                    mm(ps, khT[:, jt * JTS:(jt + 1) * JTS],
                       q_sl[:, it * ST:(it + 1) * ST], start=True, stop=True)
                    pt = ptile.tile([128, ST], MDT, tag="pt")
                    nc.scalar.activation(out=pt, in_=ps, func=Exp, scale=0.125)
                    if it == it0:
                        o = jt - 4 * it0
                        nc.vector.tensor_mul(
                            pt, pt, cm_sb[:, 384 - 128 * o:896 - 128 * o])
                    mm(po[it], vh_aug[:, jt, :], pt,
                       start=(jt == 0), stop=(jt == 4 * it + 3))
            # pack the 4 softmax denominators on quadrant rows {0,32} x 2 spans
            sums = small.tile([33, 2 * ST], f32, tag="sums")
            rc = small.tile([33, 2 * ST], MDT, tag="rc")
            nc.vector.memset(sums, 1.0)
            for it in range(NS):
                r0, c0 = 32 * (it // 2), ST * (it % 2)
                nc.scalar.copy(sums[r0:r0 + 1, c0:c0 + ST], po[it][64:65, :])
            with nc.allow_low_precision(reason="f32r recip out, same width as f32"):
                nc.vector.reciprocal(rc, sums)
            pe_touch(rc[:, 0:1])
            for it in range(NS):
                r0, c0 = 32 * (it // 2), ST * (it % 2)
                a_sl = attn[h * 64:(h + 1) * 64, it * ST:(it + 1) * ST]
                nc.scalar.copy(a_sl, po[it][0:64, :])
                bcp = psum.tile([64, ST], f32, tag="mm", bufs=3, name="bcp")
                mm(bcp, ones4q[r0:r0 + 1, :],
                   rc[r0:r0 + 1, c0:c0 + ST], start=True, stop=True)
                nc.vector.tensor_mul(a_sl, a_sl, bcp)

        # ---- output projection (partial over this core's 128 dims) ----
        for e in range(NE):
            for it in range(NS):
                pw = psum.tile([128, ST], f32, tag="mm", bufs=3, name="pw")
                mm(pw, wo_sb[:, e * 128:(e + 1) * 128],
                   attn[:, it * ST:(it + 1) * ST], start=True, stop=True)
                ob = outb.tile([128, ST], f32, tag="ob")
                nc.vector.tensor_copy(ob, pw)
                nc.sync.dma_start(
                    out=outT[e * 128:(e + 1) * 128, it * ST:(it + 1) * ST], in_=ob
                )

    nc.compile()
    _CACHE["nc"] = nc
    return nc


def _host_tables():
    if "tables" in _CACHE:
        return _CACHE["tables"]
    # faithful to reference: exp = -2*arange(0,64,2)/64
    expv = -2.0 * np.arange(0, HD, 2, dtype=np.float32) / HD
    thetas = np.power(np.float32(10000.0), expv).astype(np.float32)    # [32]
    m = np.arange(S, dtype=np.float32)
    freq = np.outer(m, thetas).astype(np.float32)                      # [S, 32]
    cos = np.cos(freq).astype(np.float32).T                            # [32, S]
    sin = np.sin(freq).astype(np.float32).T
    cos64 = np.concatenate([cos, cos], 0)                              # [64, S]
    sin64 = np.concatenate([-sin, sin], 0)                             # [64, S]
    cos64 = np.ascontiguousarray(cos64)
    sin64 = np.ascontiguousarray(sin64)
    perm = np.concatenate([np.arange(0, HD, 2), np.arange(1, HD, 2)])  # deinterleave
    slide = (np.arange(128)[:, None] <= (np.arange(896)[None, :] - 384))
    slide = np.ascontiguousarray(slide.astype(np.float32))
    _CACHE["tables"] = (cos64, sin64, perm, slide)
    return _CACHE["tables"]


def kernel(**inputs):
    q = np.asarray(inputs["q"], np.float32)[0]       # [S, D]
    k = np.asarray(inputs["k"], np.float32)[0]
    v = np.asarray(inputs["v"], np.float32)[0]
    Wq = np.asarray(inputs["Wq"], np.float32)
    Wk = np.asarray(inputs["Wk"], np.float32)
    Wv = np.asarray(inputs["Wv"], np.float32)
    Wo = np.asarray(inputs["Wo"], np.float32)
    bq = np.asarray(inputs["bq"], np.float32)
    bk = np.asarray(inputs["bk"], np.float32)
    bv = np.asarray(inputs["bv"], np.float32)
    bo = np.asarray(inputs["bo"], np.float32)

    cos64, sin64, perm, slide = _host_tables()

    # head_dim deinterleave permutation applied to q/k projection columns
    permQ = np.concatenate([h * HD + perm for h in range(HEADS)])
    permK = np.concatenate([g * HD + perm for g in range(KVH)])
    Wqp = Wq[:, permQ]
    bqp = bq[permQ]
    Wkp = Wk[:, permK]
    bkp = bk[permK]

    qT = np.ascontiguousarray(q.T)
    kT = np.ascontiguousarray(k.T)
    vT = np.ascontiguousarray(v.T)

    in_maps = []
    head_pairs = []
    for c in range(N_CORES):
        g = c // 2
        if c % 2 == 0:
            h0, h1 = g, g + 4
        else:
            h0, h1 = g + 8, g + 12
        head_pairs.append((h0, h1))
        wq_c = np.ascontiguousarray(
            np.concatenate([Wqp[:, h0 * HD:(h0 + 1) * HD],
                            Wqp[:, h1 * HD:(h1 + 1) * HD]], axis=1))
        bq_c = np.ascontiguousarray(
            np.concatenate([bqp[h0 * HD:(h0 + 1) * HD],
                            bqp[h1 * HD:(h1 + 1) * HD]]).reshape(128, 1))
        wo_c = np.ascontiguousarray(
            np.concatenate([Wo[h0 * HD:(h0 + 1) * HD, :],
                            Wo[h1 * HD:(h1 + 1) * HD, :]], axis=0))
        in_maps.append({
            "qT": qT, "kT": kT, "vT": vT,
            "wq": wq_c,
            "wk": np.ascontiguousarray(Wkp[:, g * HD:(g + 1) * HD]),
            "wv": np.ascontiguousarray(Wv[:, g * HD:(g + 1) * HD]),
            "wo": wo_c,
            "bq": bq_c,
            "bk": np.ascontiguousarray(bkp[g * HD:(g + 1) * HD].reshape(64, 1)),
            "bv": np.ascontiguousarray(bv[g * HD:(g + 1) * HD].reshape(64, 1)),
            "cosk": cos64, "sink": sin64, "cmask": slide,
            "ident": np.eye(64, dtype=np.float32),
            "ones": np.ones((128, 64), np.float32),
        })

    nc = _build_program()
    res = run_bass_kernel_spmd(nc, in_maps, list(range(N_CORES)))
    acc = np.zeros((D, S), np.float32)
    for r in res.results:
        acc += np.asarray(r["outT"], np.float32)
    out = acc.T + bo[None, :]
    return out[None].astype(np.float32)
